# revision 1
# baseline (speedup 1.0000x reference)
# Trainium2 Bass kernel for nn_CompressedGPT2Attention.
#
# Model: B=2, S=2048, D=1024, H=16 heads of HD=64.
#   qkv = x @ c_attn_w + c_attn_b ; causal attention per head;
#   per-head symmetric projector on the attention output; out = attn @ c_proj_w + b.
#
# Sharding (megatron-style tensor parallel over heads, 8 cores x 2 heads):
#   - every core gets the full hidden_states
#   - c_attn (q,k,v) columns + projectors + c_proj rows are sharded by head
#   - each core computes a full-shape partial of the c_proj output; the
#     all-reduce after c_proj is done on the host (partials are summed there).
#
# On-core layout strategy: activations are kept feature-major ("transposed",
# features on SBUF partitions) so every matmul contracts over the partition
# dim without ever transposing big intermediates:
#   xT[d, s]   provided by the host (input marshalling) and cast to bf16
#              by the SWDGE load DMA
#   qT,kT[f,s] = W^T @ xT
#   scoresT[kj, qi] = kT^T-slice matmuls (two heads packed on the PE via
#                     tile_position row-tiling, K=64 each)
#   expT = exp(scoresT/8) on ScalarE, causal mask via gpsimd affine_select
#   v[s, hd]   computed directly in sequence-major layout (xT stationary)
#   attn_unT[hd, qi] accumulated over kj with lhsT = v; softmax sums
#                     ride along as a concurrent ones-column matmul
#   attnP_T[e, qi]  = projector matmul (two heads packed, K=64)
#   normalization   = (1/sums) broadcast across partitions with a K=1 matmul,
#                     then one VectorE multiply
#   outT[dout, s]   = c_proj partial, written back fp32; host sums over cores.

import numpy as np

B, S, D, H, HD = 2, 2048, 1024, 16, 64
BS = B * S
N_CORES = 8
HPC = H // N_CORES  # heads per core = 2

_CACHE = {}
USE_CRIT = False  # tile_critical around paired matmuls faults the device


def _build(nc):
    import concourse.bass as bass
    import concourse.mybir as mybir
    import concourse.tile as tile
    from contextlib import ExitStack

    f32 = mybir.dt.float32
    bf16 = mybir.dt.bfloat16
    AF = mybir.ActivationFunctionType
    OP = mybir.AluOpType

    x_d = nc.dram_tensor("xT", [D, BS], bf16, kind="ExternalInput").ap()
    wqk_d = nc.dram_tensor("w_qk", [D, 2 * HPC * HD], bf16, kind="ExternalInput").ap()
    wv_d = nc.dram_tensor("w_v", [D, HPC * HD], bf16, kind="ExternalInput").ap()
    bqk_d = nc.dram_tensor("b_qk", [2 * HPC * HD], f32, kind="ExternalInput").ap()
    bv_d = nc.dram_tensor("b_v", [HPC * HD], f32, kind="ExternalInput").ap()
    wpr_d = nc.dram_tensor("w_pr", [HPC * HD, HD], bf16, kind="ExternalInput").ap()
    wcp_d = nc.dram_tensor("w_cp", [HPC * HD, D], bf16, kind="ExternalInput").ap()
    bcp_d = nc.dram_tensor("b_cp", [D], f32, kind="ExternalInput").ap()
    out_d = nc.dram_tensor("outT", [8, 128, BS], f32, kind="ExternalOutput").ap()

    F = HPC * HD  # 128 features per block (2 heads stacked)
    NB = BS // 512  # 8 s-blocks of 512
    KT = D // 128  # 8 contraction tiles

    from contextlib import nullcontext

    with TileCtx(tile, nc) as tc:
        crit = (lambda: tc.tile_critical()) if USE_CRIT else (lambda: nullcontext())
        # ---------------- persistent tiles ----------------
        # tc.tile singles must be released in LIFO order, and their free
        # closures must be kept alive (GC of a discarded closure releases
        # the pool at a random trace point). xT goes last so it can be
        # freed right after the qkv phase.
        frees = []

        def ptile(shape, dtype, name):
            t, free = tc.tile(shape, dtype, name=name)
            frees.append(free)
            return t

        qT = ptile([128, BS], bf16, "qT")
        kTt = ptile([128, BS], bf16, "kTt")
        v_s = ptile([128, BS // 128, 128], bf16, "v_s")
        wqk_sb = ptile([128, KT, 2 * F], bf16, "wqk_sb")
        wv_sb = ptile([128, KT, F], bf16, "wv_sb")
        wpr_sb = ptile([128, HD], bf16, "wpr_sb")
        wcp_sb = ptile([128, D], bf16, "wcp_sb")
        bqk_sb = ptile([128, 2], f32, "bqk_sb")
        bcp_sb = ptile([128, 8], f32, "bcp_sb")
        ones_w = ptile([128, 64], bf16, "ones_w")
        ones_row = ptile([1, 128], bf16, "ones_row")
        bv16 = ptile([1, 128], bf16, "bv16")
        bias_v_bc = ptile([128, 128], f32, "bias_v_bc")
        # one tile per 512-wide s-block so c_proj can start per-block
        cpr = [ptile([128, 512], bf16, f"cpr{i}") for i in range(NB)]

        xT, xT_free = tc.tile([128, KT, BS], bf16, name="xT")

        # ---------------- constants + weights ----------------
        nc.any.memset(ones_w[:], 1.0)
        nc.any.memset(ones_row[:], 1.0)
        nc.sync.dma_start(wqk_sb[:], wqk_d.rearrange("(kt p) f -> p kt f", p=128))
        nc.sync.dma_start(wv_sb[:], wv_d.rearrange("(kt p) f -> p kt f", p=128))
        nc.sync.dma_start(wpr_sb[:], wpr_d)
        nc.sync.dma_start(wcp_sb[:], wcp_d)
        nc.sync.dma_start(bqk_sb[:], bqk_d.rearrange("(t p) -> p t", p=128))
        nc.gpsimd.dma_start(bv16[:], bv_d[None, :])
        nc.sync.dma_start(bcp_sb[:], bcp_d.rearrange("(t p) -> p t", p=128))

        # ---------------- load x^T (bf16, marshalled on the host) -------------
        with ExitStack() as phase1:
            for kt in range(KT):
                nc.sync.dma_start(
                    xT[:, kt, :], x_d[kt * 128 : (kt + 1) * 128, :]
                )

            qkv_ps = phase1.enter_context(
                tc.tile_pool(name="qkv_ps", bufs=3, space="PSUM")
            )
            vt_ps = phase1.enter_context(
                tc.tile_pool(name="vt_ps", bufs=1, space="PSUM")
            )

            # ---------------- q^T / k^T matmuls ----------------
            for ft in range(2):  # 0=q, 1=k
                dest = (qT, kTt)[ft]
                for sb in range(NB):
                    ps = qkv_ps.tile([128, 512], f32, tag="qkv")
                    for kt in range(KT):
                        nc.tensor.matmul(
                            ps[:],
                            wqk_sb[:, kt, ft * F : (ft + 1) * F],
                            xT[:, kt, sb * 512 : (sb + 1) * 512],
                            start=(kt == 0),
                            stop=(kt == KT - 1),
                        )
                    nc.scalar.activation(
                        dest[:, sb * 512 : (sb + 1) * 512], ps[:], AF.Identity,
                        bias=bqk_sb[:, ft : ft + 1],
                    )

            # ---------------- v bias broadcast tile ----------------
            # v is produced sequence-major, so its per-feature bias lives on
            # the free dim; build [128,128] tile with every row = b_v once
            # (K=1 matmul against ones) and add it during the psum drain.
            ps_bv = vt_ps.tile([128, 128], f32, tag="vt")
            nc.tensor.matmul(
                ps_bv[:], ones_row[:], bv16[:],
                start=True, stop=True,
            )
            nc.vector.tensor_copy(bias_v_bc[:], ps_bv[:])

            # ---------------- v, directly in sequence-major [s, hd] ----------
            for st in range(BS // 128):
                ps_v = qkv_ps.tile([128, 128], f32, tag="v")
                for kt in range(KT):
                    nc.tensor.matmul(
                        ps_v[:],
                        xT[:, kt, st * 128 : (st + 1) * 128],
                        wv_sb[:, kt, :],
                        start=(kt == 0),
                        stop=(kt == KT - 1),
                    )
                nc.vector.scalar_tensor_tensor(
                    v_s[:, st, :], ps_v[:], 1.0, bias_v_bc[:],
                    OP.mult, OP.add,
                )
        xT_free()

        # ---------------- attention ----------------
        with ExitStack() as phase2:
            sc_ps = phase2.enter_context(tc.tile_pool(name="sc_ps", bufs=4, space="PSUM"))
            attn_ps = phase2.enter_context(tc.tile_pool(name="attn_ps", bufs=2, space="PSUM"))
            aux_ps = phase2.enter_context(tc.tile_pool(name="aux_ps", bufs=2, space="PSUM"))
            epool = phase2.enter_context(tc.tile_pool(name="epool", bufs=6))
            spool = phase2.enter_context(tc.tile_pool(name="spool", bufs=2))
            opool = phase2.enter_context(tc.tile_pool(name="opool", bufs=4))

            for qt in range(4):
                for b in range(B):
                    blk = b * 4 + qt
                    qi = b * S + qt * 512
                    nkj = 4 * (qt + 1)
                    ps_attn = attn_ps.tile([128, 512], f32, tag="attn")
                    ps_sums = aux_ps.tile([128, 512], f32, tag="aux")
                    for kj in range(nkj):
                        kjc = b * S + kj * 128
                        p = kj - 4 * qt
                        pscA = sc_ps.tile([128, 512], f32, tag="sc")
                        pscB = sc_ps.tile([128, 512], f32, tag="sc")
                        nc.tensor.matmul(
                            pscA[:], kTt[0:64, kjc : kjc + 128],
                            qT[0:64, qi : qi + 512],
                            start=True, stop=True, tile_position=(0, 0),
                        )
                        nc.tensor.matmul(
                            pscB[:], kTt[64:128, kjc : kjc + 128],
                            qT[64:128, qi : qi + 512],
                            start=True, stop=True, tile_position=(64, 0),
                        )
                        eA = epool.tile([128, 512], bf16, tag="e")
                        eB = epool.tile([128, 512], bf16, tag="e")
                        for e, psc in ((eA, pscA), (eB, pscB)):
                            if p > 0:
                                # fully-masked left region: zero instead of exp
                                nc.gpsimd.memset(e[:, 0 : 128 * p], 0.0)
                                nc.scalar.activation(
                                    e[:, 128 * p : 512], psc[:, 128 * p : 512],
                                    AF.Exp, scale=0.125,
                                )
                            else:
                                nc.scalar.activation(e[:], psc[:], AF.Exp, scale=0.125)
                            if p >= 0:
                                # triangle mask on the 128-wide diagonal square
                                nc.gpsimd.affine_select(
                                    e[:, 128 * p : 128 * (p + 1)],
                                    e[:, 128 * p : 128 * (p + 1)],
                                    pattern=[[1, 128]], base=0,
                                    channel_multiplier=-1,
                                    compare_op=OP.is_ge, fill=0.0,
                                )
                        first, last = kj == 0, kj == nkj - 1
                        vs = v_s[:, b * 16 + kj, :]
                        nc.tensor.matmul(
                            ps_attn[0:64, :], vs[:, 0:64], eA[:],
                            start=first, stop=last, tile_position=(0, 0),
                            skip_group_check=True,
                        )
                        nc.tensor.matmul(
                            ps_attn[64:128, :], vs[:, 64:128], eB[:],
                            start=first, stop=last, tile_position=(0, 64),
                            skip_group_check=True,
                        )
                        nc.tensor.matmul(
                            ps_sums[0:64, :], ones_w[:, 0:64], eA[:],
                            start=first, stop=last, tile_position=(0, 0),
                            skip_group_check=True,
                        )
                        nc.tensor.matmul(
                            ps_sums[64:128, :], ones_w[:, 0:64], eB[:],
                            start=first, stop=last, tile_position=(0, 64),
                            skip_group_check=True,
                        )

                    attn_sb = spool.tile([128, 512], bf16, tag="attn_sb")
                    nc.vector.tensor_copy(attn_sb[:], ps_attn[:])
                    # sums are matmul-broadcast across partitions, so one
                    # DVE reciprocal yields the normalization tile directly
                    rec_bc = spool.tile([128, 512], f32, tag="rec_bc")
                    nc.vector.reciprocal(rec_bc[:], ps_sums[:])

                    ps_attnP = attn_ps.tile([128, 512], f32, tag="attn")
                    with crit():
                        nc.tensor.matmul(
                            ps_attnP[0:64, :], wpr_sb[0:64, :], attn_sb[0:64, :],
                            start=True, stop=True, tile_position=(0, 0),
                            skip_group_check=True,
                        )
                        nc.tensor.matmul(
                            ps_attnP[64:128, :], wpr_sb[64:128, :], attn_sb[64:128, :],
                            start=True, stop=True, tile_position=(64, 64),
                            skip_group_check=True,
                        )
                    nc.vector.tensor_tensor(
                        cpr[blk][:], ps_attnP[:], rec_bc[:], OP.mult
                    )

                    # ---- c_proj for this s-block, interleaved with attention
                    for dt in range(8):
                        pcp = aux_ps.tile([128, 512], f32, tag="aux")
                        nc.tensor.matmul(
                            pcp[:], wcp_sb[:, dt * 128 : (dt + 1) * 128],
                            cpr[blk][:], start=True, stop=True,
                        )
                        ot = opool.tile([128, 512], f32, tag="ot")
                        if dt % 2 == 0:
                            nc.scalar.activation(
                                ot[:], pcp[:], AF.Identity,
                                bias=bcp_sb[:, dt : dt + 1],
                            )
                        else:
                            nc.vector.tensor_scalar(
                                ot[:], pcp[:], bcp_sb[:, dt : dt + 1], None, OP.add
                            )
                        nc.sync.dma_start(
                            out_d[dt][:, blk * 512 : (blk + 1) * 512], ot[:]
                        )

        for free in reversed(frees):
            free()


class TileCtx:
    """Thin helper so _build can use `tc.tile` / `tc.tile_pool` uniformly."""

    def __init__(self, tile_mod, nc):
        self._tc = tile_mod.TileContext(nc)

    def __enter__(self):
        self._tc.__enter__()
        return self._tc

    def __exit__(self, *exc):
        return self._tc.__exit__(*exc)


def _shard_inputs(inputs):
    import ml_dtypes

    bf = ml_dtypes.bfloat16
    # host-side input marshalling: transpose of hidden_states + bf16 rounding
    # (identical to the on-device SWDGE cast) for the matmul operands
    xT = np.ascontiguousarray(
        np.asarray(inputs["hidden_states"], dtype=np.float32).reshape(BS, D).T
    ).astype(bf)
    Wa = np.asarray(inputs["c_attn_w"], dtype=np.float32)
    ba = np.asarray(inputs["c_attn_b"], dtype=np.float32)
    Wp = np.asarray(inputs["c_proj_w"], dtype=np.float32)
    bp = np.asarray(inputs["c_proj_b"], dtype=np.float32)
    proj = np.asarray(inputs["projectors"], dtype=np.float32)

    in_maps = []
    F = HPC * HD
    for c in range(N_CORES):
        sl = slice(c * F, (c + 1) * F)
        in_maps.append(
            {
                "xT": xT,
                "w_qk": np.ascontiguousarray(
                    np.concatenate([Wa[:, sl], Wa[:, D + c * F : D + (c + 1) * F]], axis=1)
                ).astype(bf),
                "w_v": np.ascontiguousarray(
                    Wa[:, 2 * D + c * F : 2 * D + (c + 1) * F]
                ).astype(bf),
                "b_qk": np.ascontiguousarray(
                    np.concatenate([ba[sl], ba[D + c * F : D + (c + 1) * F]])
                ),
                "b_v": np.ascontiguousarray(ba[2 * D + c * F : 2 * D + (c + 1) * F]),
                "w_pr": np.ascontiguousarray(
                    proj[HPC * c : HPC * (c + 1)].reshape(F, HD)
                ).astype(bf),
                "w_cp": np.ascontiguousarray(Wp[sl, :]).astype(bf),
                "b_cp": bp if c == 0 else np.zeros_like(bp),
            }
        )
    return in_maps


def _get_nc():
    if "nc" not in _CACHE:
        from concourse import bacc

        nc = bacc.Bacc("TRN2", debug=False, num_devices=N_CORES)
        _build(nc)
        # Bacc.compile() runs generate_event_semaphores, which spills
        # per-instruction sync waits beyond the single HW wait slot into
        # separate EventSemaphore instructions — without it walrus fails
        # with "Too many sync wait commands".
        nc.compile()
        _CACHE["nc"] = nc
    return _CACHE["nc"]


def _run(inputs, trace=False, trace_kwargs=None):
    from concourse.bass_utils import run_bass_kernel_spmd

    nc = _get_nc()
    in_maps = _shard_inputs(inputs)
    res = run_bass_kernel_spmd(
        nc,
        in_maps,
        core_ids=list(range(N_CORES)),
        trace=trace,
        **(trace_kwargs or {}),
    )
    acc = np.zeros((8, 128, BS), dtype=np.float32)
    for r in res.results:
        acc += np.asarray(r["outT"], dtype=np.float32)
    out = acc.transpose(2, 0, 1).reshape(BS, D).reshape(B, S, D)
    return np.ascontiguousarray(out), res


def kernel(**inputs) -> np.ndarray:
    out, _ = _run(inputs, trace=False)
    return out


def simulate_core(inputs, core=0):
    """CoreSim one core's program (for correctness debugging). Returns outT."""
    from concourse.bass_interp import CoreSim

    nc = _get_nc()
    in_maps = _shard_inputs(inputs)
    sim = CoreSim(nc, trace=False)
    for name, arr in in_maps[core].items():
        sim.tensor(name)[:] = arr
    sim.simulate()
    return np.array(sim.tensor("outT"))



# revision 3
# speedup vs baseline: 1.2186x; 1.2186x over previous
# Trainium2 Bass kernel for nn_CompressedGPT2Attention.
#
# Model: B=2, S=2048, D=1024, H=16 heads of HD=64.
#   qkv = x @ c_attn_w + c_attn_b ; causal attention per head;
#   per-head symmetric projector on the attention output; out = attn @ c_proj_w + b.
#
# Sharding (megatron-style tensor parallel over heads, 8 cores x 2 heads):
#   - every core gets the full hidden_states
#   - c_attn (q,k,v) columns sharded by head; the per-head projector is folded
#     into c_proj on the host (W~_h = proj_h @ c_proj_rows_h), so each core's
#     output weight is [128, 1024] exactly like a plain c_proj row shard
#   - each core writes a full-shape bf16 partial of the output; the all-reduce
#     after c_proj is done on the host (f32 sum + bias there).
#
# v2 structure (vs the 277us baseline):
#   - phase 1 streams x in 512-column chunks (double buffered) so the PE
#     starts ~3us in and stays HAM-warm
#   - phase 2 is software-pipelined with lag 1: scores(i) then attn(i-1),
#     so the exp on ScalarE overlaps PE work instead of ping-ponging
#   - exp is one [128,1024] ACT per kj step (both heads, PSUM pair spanning
#     2 banks)
#   - softmax reciprocal via ScalarE Ln then Exp(scale=-1) (same act table
#     set as exp -> no table switches), freeing the DVE RECIPROCAL 3.3us/blk
#   - c_proj matmuls for a finished block are spread 2-per-step into the
#     following steps so they never stall on normalization.

import numpy as np

B, S, D, H, HD = 2, 2048, 1024, 16, 64
BS = B * S
N_CORES = 8
HPC = H // N_CORES  # heads per core = 2

_CACHE = {}


def _build(nc):
    import concourse.bass as bass
    import concourse.mybir as mybir
    import concourse.tile as tile
    from contextlib import ExitStack

    f32 = mybir.dt.float32
    bf16 = mybir.dt.bfloat16
    AF = mybir.ActivationFunctionType
    OP = mybir.AluOpType

    x_d = nc.dram_tensor("xT", [D, BS], bf16, kind="ExternalInput").ap()
    wqk_d = nc.dram_tensor("w_qk", [D, 2 * HPC * HD], bf16, kind="ExternalInput").ap()
    wv_d = nc.dram_tensor("w_v", [D, HPC * HD], bf16, kind="ExternalInput").ap()
    bqk_d = nc.dram_tensor("b_qk", [2 * HPC * HD], f32, kind="ExternalInput").ap()
    bv_d = nc.dram_tensor("b_v512", [512], f32, kind="ExternalInput").ap()
    wtil_d = nc.dram_tensor("w_til", [HPC * HD, D], bf16, kind="ExternalInput").ap()
    out_d = nc.dram_tensor("outT", [8, 128, BS], bf16, kind="ExternalOutput").ap()

    F = HPC * HD  # 128 features per block (2 heads stacked)
    NB = BS // 512  # 8 s-blocks of 512
    KT = D // 128  # 8 contraction tiles

    with TileCtx(tile, nc) as tc:
        frees = []

        def ptile(shape, dtype, name):
            t, free = tc.tile(shape, dtype, name=name)
            frees.append(free)
            return t

        qT = ptile([128, BS], bf16, "qT")
        kTt = ptile([128, BS], bf16, "kTt")
        v_s = ptile([128, BS // 128, 128], bf16, "v_s")
        wqk_sb = ptile([128, KT, 2 * F], bf16, "wqk_sb")
        wv_sb = ptile([128, KT, F], bf16, "wv_sb")
        wtil_sb = ptile([128, D], bf16, "wtil_sb")
        bqk_sb = ptile([128, 2], f32, "bqk_sb")
        ones_w = ptile([128, 64], bf16, "ones_w")
        ones_row = ptile([1, 128], bf16, "ones_row")
        bv16 = ptile([1, 512], bf16, "bv16")
        bias_v_bc = ptile([128, 512], f32, "bias_v_bc")
        cpr = [ptile([128, 512], bf16, f"cpr{i}") for i in range(NB)]

        # ---------------- constants + weights ----------------
        nc.any.memset(ones_w[:], 1.0)
        nc.any.memset(ones_row[:], 1.0)
        nc.sync.dma_start(wqk_sb[:], wqk_d.rearrange("(kt p) f -> p kt f", p=128))
        nc.sync.dma_start(wv_sb[:], wv_d.rearrange("(kt p) f -> p kt f", p=128))
        nc.sync.dma_start(wtil_sb[:], wtil_d)
        nc.sync.dma_start(bqk_sb[:], bqk_d.rearrange("(t p) -> p t", p=128))
        nc.gpsimd.dma_start(bv16[:], bv_d[None, :])

        # ---------------- phase 1: stream x, compute q/k/v ----------------
        with ExitStack() as phase1:
            ch_pool = phase1.enter_context(tc.tile_pool(name="xchunk", bufs=2))
            qkv_ps = phase1.enter_context(
                tc.tile_pool(name="qkv_ps", bufs=3, space="PSUM")
            )

            # v bias broadcast tile: [128,512] rows all equal to tiled b_v
            ps_bv = qkv_ps.tile([128, 512], f32, tag="q")
            nc.tensor.matmul(ps_bv[:], ones_row[:], bv16[:], start=True, stop=True)
            nc.vector.tensor_copy(bias_v_bc[:], ps_bv[:])

            for sb in range(NB):
                chunk = ch_pool.tile([128, KT, 512], bf16, tag="x")
                nc.sync.dma_start(
                    chunk[:],
                    x_d[:, sb * 512 : (sb + 1) * 512].rearrange(
                        "(kt p) s -> p kt s", p=128
                    ),
                )
                # q and k (feature-major), one psum chain each
                for ft in range(2):
                    dest = (qT, kTt)[ft]
                    ps = qkv_ps.tile([128, 512], f32, tag="q")
                    for kt in range(KT):
                        nc.tensor.matmul(
                            ps[:],
                            wqk_sb[:, kt, ft * F : (ft + 1) * F],
                            chunk[:, kt, :],
                            start=(kt == 0),
                            stop=(kt == KT - 1),
                        )
                    nc.vector.tensor_scalar(
                        dest[:, sb * 512 : (sb + 1) * 512], ps[:],
                        bqk_sb[:, ft : ft + 1], None, OP.add,
                    )
                # v, sequence-major: 4 st-tiles of 128 into one psum bank
                ps_v = qkv_ps.tile([128, 512], f32, tag="q")
                for j in range(4):
                    for kt in range(KT):
                        nc.tensor.matmul(
                            ps_v[:, j * 128 : (j + 1) * 128],
                            chunk[:, kt, j * 128 : (j + 1) * 128],
                            wv_sb[:, kt, :],
                            start=(kt == 0),
                            stop=(kt == KT - 1),
                            skip_group_check=True,
                        )
                nc.vector.scalar_tensor_tensor(
                    v_s[:, sb * 4 : (sb + 1) * 4, :], ps_v[:], 1.0, bias_v_bc[:],
                    OP.mult, OP.add,
                )

        # ---------------- phase 2: attention, pipelined ----------------
        with ExitStack() as phase2:
            sc_ps = phase2.enter_context(tc.tile_pool(name="sc_ps", bufs=2, space="PSUM"))
            attn_ps = phase2.enter_context(tc.tile_pool(name="attn_ps", bufs=1, space="PSUM"))
            sums_ps = phase2.enter_context(tc.tile_pool(name="sums_ps", bufs=1, space="PSUM"))
            cp_ps = phase2.enter_context(tc.tile_pool(name="cp_ps", bufs=2, space="PSUM"))
            epool = phase2.enter_context(tc.tile_pool(name="epool", bufs=3))
            upool = phase2.enter_context(tc.tile_pool(name="upool", bufs=2))
            rpool = phase2.enter_context(tc.tile_pool(name="rpool", bufs=2))
            opool = phase2.enter_context(tc.tile_pool(name="opool", bufs=4))

            # step list: blocks in (batch, qt) order; kj tiles of 128 keys
            steps = []
            for b in range(B):
                for qt in range(4):
                    nkj = 4 * (qt + 1)
                    for kj in range(nkj):
                        steps.append((b, qt, kj, kj == 0, kj == nkj - 1))

            state = {}  # live attn/sums psum + e tiles keyed by step idx
            pend_cproj = []

            def emit_scores(i):
                b, qt, kj, first, last = steps[i]
                qi = b * S + qt * 512
                kjc = b * S + kj * 128
                psc = sc_ps.tile([128, 1024], f32, tag="sc")
                nc.tensor.matmul(
                    psc[:, 0:512], kTt[0:64, kjc : kjc + 128],
                    qT[0:64, qi : qi + 512],
                    start=True, stop=True, tile_position=(0, 0),
                )
                nc.tensor.matmul(
                    psc[:, 512:1024], kTt[64:128, kjc : kjc + 128],
                    qT[64:128, qi : qi + 512],
                    start=True, stop=True, tile_position=(64, 0),
                )
                # exp (ScalarE) + causal masking (GpSimd)
                p = kj - 4 * qt
                e = epool.tile([128, 1024], bf16, tag="e")
                if p > 0:
                    nc.gpsimd.memset(e[:, 0 : 128 * p], 0.0)
                    nc.gpsimd.memset(e[:, 512 : 512 + 128 * p], 0.0)
                    nc.scalar.activation(
                        e[:, 128 * p : 512], psc[:, 128 * p : 512], AF.Exp, scale=0.125
                    )
                    nc.scalar.activation(
                        e[:, 512 + 128 * p : 1024], psc[:, 512 + 128 * p : 1024],
                        AF.Exp, scale=0.125,
                    )
                else:
                    nc.scalar.activation(e[:], psc[:], AF.Exp, scale=0.125)
                if p >= 0:
                    for off in (0, 512):
                        nc.gpsimd.affine_select(
                            e[:, off + 128 * p : off + 128 * (p + 1)],
                            e[:, off + 128 * p : off + 128 * (p + 1)],
                            pattern=[[1, 128]], base=0,
                            channel_multiplier=-1,
                            compare_op=OP.is_ge, fill=0.0,
                        )
                state[i] = e

            def emit_attn(i):
                b, qt, kj, first, last = steps[i]
                e = state.pop(i)
                if first:
                    state["attn"] = attn_ps.tile([128, 512], f32, tag="attn", name="ps_attn")
                    state["sums"] = sums_ps.tile([128, 512], f32, tag="sums", name="ps_sums")
                ps_attn, ps_sums = state["attn"], state["sums"]
                vs = v_s[:, b * 16 + kj, :]
                eA, eB = e[:, 0:512], e[:, 512:1024]
                nc.tensor.matmul(
                    ps_attn[0:64, :], vs[:, 0:64], eA,
                    start=first, stop=last, tile_position=(0, 0),
                    skip_group_check=True,
                )
                nc.tensor.matmul(
                    ps_attn[64:128, :], vs[:, 64:128], eB,
                    start=first, stop=last, tile_position=(0, 64),
                    skip_group_check=True,
                )
                nc.tensor.matmul(
                    ps_sums[0:64, :], ones_w[:, 0:64], eA,
                    start=first, stop=last, tile_position=(0, 0),
                    skip_group_check=True,
                )
                nc.tensor.matmul(
                    ps_sums[64:128, :], ones_w[:, 0:64], eB,
                    start=first, stop=last, tile_position=(0, 64),
                    skip_group_check=True,
                )
                if last:
                    blk = b * 4 + qt
                    ps_attn = state.pop("attn")
                    ps_sums = state.pop("sums")
                    # un-normalized attn out of PSUM fast (frees the bank)
                    unA = upool.tile([128, 512], bf16, tag="u")
                    nc.vector.tensor_copy(unA[:], ps_attn[:])
                    # 1/sums on ScalarE: exp(-ln(x)); same act table set as exp
                    ln_t = rpool.tile([128, 512], f32, tag="r")
                    nc.scalar.activation(ln_t[:], ps_sums[:], AF.Ln)
                    rec = rpool.tile([128, 512], f32, tag="r")
                    nc.scalar.activation(rec[:], ln_t[:], AF.Exp, scale=-1.0)
                    nc.vector.tensor_tensor(cpr[blk][:], unA[:], rec[:], OP.mult)
                    for dt in range(8):
                        pend_cproj.append((blk, dt))

            def emit_cproj(n):
                for _ in range(n):
                    if not pend_cproj:
                        return
                    blk, dt = pend_cproj.pop(0)
                    pcp = cp_ps.tile([128, 512], f32, tag="cp")
                    nc.tensor.matmul(
                        pcp[:], wtil_sb[:, dt * 128 : (dt + 1) * 128],
                        cpr[blk][:], start=True, stop=True,
                    )
                    ot = opool.tile([128, 512], bf16, tag="ot")
                    nc.vector.tensor_copy(ot[:], pcp[:])
                    nc.sync.dma_start(
                        out_d[dt][:, blk * 512 : (blk + 1) * 512], ot[:]
                    )

            # software pipeline, lag 1
            for i in range(len(steps)):
                emit_scores(i)
                if i > 0:
                    emit_attn(i - 1)
                emit_cproj(2)
            emit_attn(len(steps) - 1)
            emit_cproj(len(pend_cproj))

        for free in reversed(frees):
            free()


class TileCtx:
    """Thin helper so _build can use `tc.tile` / `tc.tile_pool` uniformly."""

    def __init__(self, tile_mod, nc):
        self._tc = tile_mod.TileContext(nc)

    def __enter__(self):
        self._tc.__enter__()
        return self._tc

    def __exit__(self, *exc):
        return self._tc.__exit__(*exc)


def _shard_inputs(inputs):
    import ml_dtypes

    bf = ml_dtypes.bfloat16
    xT = np.ascontiguousarray(
        np.asarray(inputs["hidden_states"], dtype=np.float32).reshape(BS, D).T
    ).astype(bf)
    Wa = np.asarray(inputs["c_attn_w"], dtype=np.float32)
    ba = np.asarray(inputs["c_attn_b"], dtype=np.float32)
    Wp = np.asarray(inputs["c_proj_w"], dtype=np.float32)
    proj = np.asarray(inputs["projectors"], dtype=np.float32)

    in_maps = []
    F = HPC * HD
    for c in range(N_CORES):
        sl = slice(c * F, (c + 1) * F)
        # fold per-head projector into the c_proj row shard
        wtil = np.einsum(
            "hde,hef->hdf",
            proj[HPC * c : HPC * (c + 1)],
            Wp[sl, :].reshape(HPC, HD, D),
        ).reshape(F, D)
        in_maps.append(
            {
                "xT": xT,
                "w_qk": np.ascontiguousarray(
                    np.concatenate(
                        [Wa[:, sl], Wa[:, D + c * F : D + (c + 1) * F]], axis=1
                    )
                ).astype(bf),
                "w_v": np.ascontiguousarray(
                    Wa[:, 2 * D + c * F : 2 * D + (c + 1) * F]
                ).astype(bf),
                "b_qk": np.ascontiguousarray(
                    np.concatenate([ba[sl], ba[D + c * F : D + (c + 1) * F]])
                ),
                "b_v512": np.ascontiguousarray(
                    np.tile(ba[2 * D + c * F : 2 * D + (c + 1) * F], 4)
                ),
                "w_til": np.ascontiguousarray(wtil).astype(bf),
            }
        )
    return in_maps


def _get_nc():
    if "nc" not in _CACHE:
        from concourse import bacc

        nc = bacc.Bacc("TRN2", debug=False, num_devices=N_CORES)
        _build(nc)
        nc.compile()
        _CACHE["nc"] = nc
    return _CACHE["nc"]


def _run(inputs, trace=False, trace_kwargs=None):
    from concourse.bass_utils import run_bass_kernel_spmd

    nc = _get_nc()
    in_maps = _shard_inputs(inputs)
    res = run_bass_kernel_spmd(
        nc,
        in_maps,
        core_ids=list(range(N_CORES)),
        trace=trace,
        **(trace_kwargs or {}),
    )
    acc = np.zeros((8, 128, BS), dtype=np.float32)
    for r in res.results:
        acc += np.asarray(r["outT"], dtype=np.float32)
    bp = np.asarray(inputs["c_proj_b"], dtype=np.float32)
    out = acc.transpose(2, 0, 1).reshape(BS, D) + bp[None, :]
    return np.ascontiguousarray(out.reshape(B, S, D)), res


def kernel(**inputs) -> np.ndarray:
    out, _ = _run(inputs, trace=False)
    return out


def simulate_core(inputs, core=0):
    """CoreSim one core's program (for correctness debugging). Returns outT."""
    from concourse.bass_interp import CoreSim

    nc = _get_nc()
    in_maps = _shard_inputs(inputs)
    sim = CoreSim(nc, trace=False)
    for name, arr in in_maps[core].items():
        sim.tensor(name)[:] = arr
    sim.simulate()
    return np.array(sim.tensor("outT"))


# revision 8
# speedup vs baseline: 1.2578x; 1.0322x over previous
# Trainium2 Bass kernel for nn_CompressedGPT2Attention.
#
# Model: B=2, S=2048, D=1024, H=16 heads of HD=64.
#   qkv = x @ c_attn_w + c_attn_b ; causal attention per head;
#   per-head symmetric projector on the attention output; out = attn @ c_proj_w + b.
#
# Sharding (megatron-style tensor parallel over heads, 8 cores x 2 heads):
#   - every core gets the full hidden_states
#   - c_attn (q,k,v) columns sharded by head; the per-head projector is folded
#     into c_proj on the host (W~_h = proj_h @ c_proj_rows_h)
#   - each core writes a full-shape bf16 partial; host does the f32 all-reduce
#     + bias.
#
# v3 structure (vs the 227us v2):
#   - qkv (phase 1) is interleaved INTO the attention step loop: sb0-3 are
#     emitted up front, sb4-7's q/k/v chains are injected between early
#     attention steps, so the PE never serializes a qkv-only prologue while
#     ScalarE idles
#   - one shared [128,512] PSUM pool serves qkv chains AND c_proj outputs
#     (2 banks) + scores 2x[128,1024] (4) + attn (1) + sums (1) = 8 banks
#   - softmax reciprocal on DVE RECIPROCAL for big blocks; ScalarE ln+exp
#     for the qt=0 blocks and the final block (keeps sums-bank hold short
#     where block gaps are tight)
#   - un-normalized attn is drained by ScalarE (Identity ACT) to free the
#     attn PSUM bank fast; normalization mult is bf16 x bf16 (2x DVE mode)
#   - c_proj matmuls are released >=4 steps after their block ends so they
#     never stall on the recip chain; drains on DVE (ScalarE in the tail)
#   - dummy exp at t=0 preloads the ACT table set during the qkv ramp

import numpy as np

B, S, D, H, HD = 2, 2048, 1024, 16, 64
BS = B * S
N_CORES = 8
HPC = H // N_CORES  # heads per core = 2

_CACHE = {}


def _build(nc):
    import concourse.bass as bass
    import concourse.mybir as mybir
    import concourse.tile as tile
    from contextlib import ExitStack

    f32 = mybir.dt.float32
    bf16 = mybir.dt.bfloat16
    AF = mybir.ActivationFunctionType
    OP = mybir.AluOpType

    x_d = nc.dram_tensor("xT", [D, BS], bf16, kind="ExternalInput").ap()
    wqk_d = nc.dram_tensor("w_qk", [D, 2 * HPC * HD], bf16, kind="ExternalInput").ap()
    wv_d = nc.dram_tensor("w_v", [D, HPC * HD], bf16, kind="ExternalInput").ap()
    bqk_d = nc.dram_tensor("b_qk", [2 * HPC * HD], f32, kind="ExternalInput").ap()
    bv_d = nc.dram_tensor("b_v512", [512], f32, kind="ExternalInput").ap()
    wtil_d = nc.dram_tensor("w_til", [HPC * HD, D], bf16, kind="ExternalInput").ap()
    out_d = nc.dram_tensor("outT", [8, 128, BS], bf16, kind="ExternalOutput").ap()

    F = HPC * HD  # 128 features (2 heads stacked)
    NB = BS // 512
    KT = D // 128

    with TileCtx(tile, nc) as tc:
        frees = []

        def ptile(shape, dtype, name):
            t, free = tc.tile(shape, dtype, name=name)
            frees.append(free)
            return t

        qT = ptile([128, BS], bf16, "qT")
        kTt = ptile([128, BS], bf16, "kTt")
        v_s = ptile([128, BS // 128, 128], bf16, "v_s")
        wqk_sb = ptile([128, KT, 2 * F], bf16, "wqk_sb")
        wv_sb = ptile([128, KT, F], bf16, "wv_sb")
        wtil_sb = ptile([128, D], bf16, "wtil_sb")
        bqk_sb = ptile([128, 2], f32, "bqk_sb")
        ones_w = ptile([128, 64], bf16, "ones_w")
        ones_row = ptile([1, 128], bf16, "ones_row")
        bv16 = ptile([1, 512], bf16, "bv16")
        bias_v_bc = ptile([128, 512], f32, "bias_v_bc")
        dummy = ptile([1, 2], f32, "dummy")
        cpr = [ptile([128, 512], bf16, f"cpr{i}") for i in range(NB)]

        nc.any.memset(ones_w[:], 1.0)
        nc.any.memset(ones_row[:], 1.0)
        nc.any.memset(dummy[:], 1.0)
        # preload the exp/ln ACT table set while the PE ramps on qkv
        nc.scalar.activation(dummy[:, 0:1], dummy[:, 1:2], AF.Exp)
        nc.scalar.activation(dummy[:, 0:1], dummy[:, 1:2], AF.Ln)

        nc.sync.dma_start(wqk_sb[:], wqk_d.rearrange("(kt p) f -> p kt f", p=128))
        nc.sync.dma_start(wv_sb[:], wv_d.rearrange("(kt p) f -> p kt f", p=128))
        nc.sync.dma_start(bqk_sb[:], bqk_d.rearrange("(t p) -> p t", p=128))
        nc.gpsimd.dma_start(bv16[:], bv_d[None, :])

        with ExitStack() as ctx:
            ch_pool = ctx.enter_context(tc.tile_pool(name="xchunk", bufs=2))
            sh_ps = ctx.enter_context(tc.tile_pool(name="sh_ps", bufs=2, space="PSUM"))
            sc_ps = ctx.enter_context(tc.tile_pool(name="sc_ps", bufs=2, space="PSUM"))
            attn_ps = ctx.enter_context(tc.tile_pool(name="attn_ps", bufs=1, space="PSUM"))
            sums_ps = ctx.enter_context(tc.tile_pool(name="sums_ps", bufs=1, space="PSUM"))
            epool = ctx.enter_context(tc.tile_pool(name="epool", bufs=3))
            upool = ctx.enter_context(tc.tile_pool(name="upool", bufs=2))
            rpool = ctx.enter_context(tc.tile_pool(name="rpool", bufs=2))
            opool = ctx.enter_context(tc.tile_pool(name="opool", bufs=4))

            # v bias broadcast tile [128,512] (rows all = tiled b_v)
            ps_bv = sh_ps.tile([128, 512], f32, tag="sh")
            nc.tensor.matmul(ps_bv[:], ones_row[:], bv16[:], start=True, stop=True)
            nc.vector.tensor_copy(bias_v_bc[:], ps_bv[:])

            chunks = {}

            def emit_chunk_dma(sb):
                chunk = ch_pool.tile([128, KT, 512], bf16, tag="x", name="chunk")
                nc.sync.dma_start(
                    chunk[:],
                    x_d[:, sb * 512 : (sb + 1) * 512].rearrange(
                        "(kt p) s -> p kt s", p=128
                    ),
                )
                chunks[sb] = chunk

            def emit_qk(sb, ft, scalar_drain):
                dest = (qT, kTt)[ft]
                ps = sh_ps.tile([128, 512], f32, tag="sh", name="ps_qk")
                for kt in range(KT):
                    nc.tensor.matmul(
                        ps[:],
                        wqk_sb[:, kt, ft * F : (ft + 1) * F],
                        chunks[sb][:, kt, :],
                        start=(kt == 0),
                        stop=(kt == KT - 1),
                    )
                dsl = dest[:, sb * 512 : (sb + 1) * 512]
                if scalar_drain:
                    nc.scalar.activation(
                        dsl, ps[:], AF.Identity, bias=bqk_sb[:, ft : ft + 1]
                    )
                else:
                    nc.vector.tensor_scalar(
                        dsl, ps[:], bqk_sb[:, ft : ft + 1], None, OP.add
                    )

            def emit_v(sb):
                ps = sh_ps.tile([128, 512], f32, tag="sh", name="ps_v")
                for j in range(4):
                    for kt in range(KT):
                        nc.tensor.matmul(
                            ps[:, j * 128 : (j + 1) * 128],
                            chunks[sb][:, kt, j * 128 : (j + 1) * 128],
                            wv_sb[:, kt, :],
                            start=(kt == 0),
                            stop=(kt == KT - 1),
                            skip_group_check=True,
                        )
                nc.vector.scalar_tensor_tensor(
                    v_s[:, sb * 4 : (sb + 1) * 4, :], ps[:], 1.0, bias_v_bc[:],
                    OP.mult, OP.add,
                )
                chunks.pop(sb)

            # ---- sb0-3 upfront (b=0 data); ScalarE does the qk drains ----
            for sb in range(4):
                emit_chunk_dma(sb)
                emit_qk(sb, 0, True)
                emit_qk(sb, 1, True)
                emit_v(sb)
            # c_proj weights are first needed ~step 5; keep their DMA out of
            # the startup critical path
            nc.sync.dma_start(wtil_sb[:], wtil_d)

            # remaining qkv work, injected between early attention steps
            inject = {}
            for n, sb in enumerate(range(4, 8)):
                base = 6 * n
                inject.setdefault(base, []).append(("dma", sb))
                inject.setdefault(base + 2, []).append(("q", sb))
                inject.setdefault(base + 4, []).append(("k", sb))
                inject.setdefault(base + 6, []).append(("v", sb))

            # ---- attention steps ----
            steps = []
            for b in range(B):
                for qt in range(4):
                    nkj = 4 * (qt + 1)
                    for kj in range(nkj):
                        steps.append((b, qt, kj, kj == 0, kj == nkj - 1))

            state = {}
            pend_cproj = []

            def emit_scores(i):
                b, qt, kj, first, last = steps[i]
                qi = b * S + qt * 512
                kjc = b * S + kj * 128
                psc = sc_ps.tile([128, 1024], f32, tag="sc", name="psc")
                nc.tensor.matmul(
                    psc[:, 0:512], kTt[0:64, kjc : kjc + 128],
                    qT[0:64, qi : qi + 512],
                    start=True, stop=True, tile_position=(0, 0),
                )
                nc.tensor.matmul(
                    psc[:, 512:1024], kTt[64:128, kjc : kjc + 128],
                    qT[64:128, qi : qi + 512],
                    start=True, stop=True, tile_position=(64, 0),
                )
                p = kj - 4 * qt
                e = epool.tile([128, 1024], bf16, tag="e", name="e")
                if p > 0:
                    nc.gpsimd.memset(e[:, 0 : 128 * p], 0.0)
                    nc.gpsimd.memset(e[:, 512 : 512 + 128 * p], 0.0)
                    nc.scalar.activation(
                        e[:, 128 * p : 512], psc[:, 128 * p : 512], AF.Exp, scale=0.125
                    )
                    nc.scalar.activation(
                        e[:, 512 + 128 * p : 1024], psc[:, 512 + 128 * p : 1024],
                        AF.Exp, scale=0.125,
                    )
                else:
                    nc.scalar.activation(e[:], psc[:], AF.Exp, scale=0.125)
                if p >= 0:
                    for off in (0, 512):
                        nc.gpsimd.affine_select(
                            e[:, off + 128 * p : off + 128 * (p + 1)],
                            e[:, off + 128 * p : off + 128 * (p + 1)],
                            pattern=[[1, 128]], base=0,
                            channel_multiplier=-1,
                            compare_op=OP.is_ge, fill=0.0,
                        )
                state[i] = e

            def emit_attn(i):
                b, qt, kj, first, last = steps[i]
                e = state.pop(i)
                if first:
                    state["attn"] = attn_ps.tile([128, 512], f32, tag="attn", name="ps_attn")
                    state["sums"] = sums_ps.tile([128, 512], f32, tag="sums", name="ps_sums")
                ps_attn, ps_sums = state["attn"], state["sums"]
                vs = v_s[:, b * 16 + kj, :]
                eA, eB = e[:, 0:512], e[:, 512:1024]
                nc.tensor.matmul(
                    ps_attn[0:64, :], vs[:, 0:64], eA,
                    start=first, stop=last, tile_position=(0, 0),
                    skip_group_check=True,
                )
                nc.tensor.matmul(
                    ps_attn[64:128, :], vs[:, 64:128], eB,
                    start=first, stop=last, tile_position=(0, 64),
                    skip_group_check=True,
                )
                nc.tensor.matmul(
                    ps_sums[0:64, :], ones_w[:, 0:64], eA,
                    start=first, stop=last, tile_position=(0, 0),
                    skip_group_check=True,
                )
                nc.tensor.matmul(
                    ps_sums[64:128, :], ones_w[:, 0:64], eB,
                    start=first, stop=last, tile_position=(0, 64),
                    skip_group_check=True,
                )
                if last:
                    blk = b * 4 + qt
                    ps_attn = state.pop("attn")
                    ps_sums = state.pop("sums")
                    # free the attn bank via ScalarE; bf16 unnormalized attn
                    unA = upool.tile([128, 512], bf16, tag="u", name="unA")
                    nc.scalar.activation(unA[:], ps_attn[:], AF.Identity)
                    rec = rpool.tile([128, 512], bf16, tag="r", name="rec")
                    with nc.allow_low_precision(reason="softmax recip in bf16"):
                        if qt == 0 or blk == NB - 1:
                            # short recip on ScalarE: 1/x = exp(-ln(x))
                            ln_t = rpool.tile([128, 512], f32, tag="r", name="ln_t")
                            nc.scalar.activation(ln_t[:], ps_sums[:], AF.Ln)
                            nc.scalar.activation(rec[:], ln_t[:], AF.Exp, scale=-1.0)
                        else:
                            nc.vector.reciprocal(rec[:], ps_sums[:])
                        nc.vector.tensor_tensor(cpr[blk][:], unA[:], rec[:], OP.mult)
                    for dt in range(8):
                        pend_cproj.append((blk, dt, i + 4))

            def emit_cproj(i, limit=2, scalar_drain=False):
                n = 0
                while pend_cproj and pend_cproj[0][2] <= i and n < limit:
                    blk, dt, _ = pend_cproj.pop(0)
                    pcp = sh_ps.tile([128, 512], f32, tag="sh", name="pcp")
                    nc.tensor.matmul(
                        pcp[:], wtil_sb[:, dt * 128 : (dt + 1) * 128],
                        cpr[blk][:], start=True, stop=True,
                    )
                    ot = opool.tile([128, 512], bf16, tag="ot", name="ot")
                    if scalar_drain and dt % 2 == 0:
                        nc.scalar.activation(ot[:], pcp[:], AF.Identity)
                    else:
                        nc.vector.tensor_copy(ot[:], pcp[:])
                    nc.sync.dma_start(
                        out_d[dt][:, blk * 512 : (blk + 1) * 512], ot[:]
                    )
                    n += 1

            for i in range(len(steps)):
                emit_scores(i)
                for item in inject.pop(i, []):
                    kind, sb = item
                    if kind == "dma":
                        emit_chunk_dma(sb)
                    elif kind == "q":
                        emit_qk(sb, 0, False)
                    elif kind == "k":
                        emit_qk(sb, 1, False)
                    else:
                        emit_v(sb)
                if i > 0:
                    emit_attn(i - 1)
                emit_cproj(i, limit=2)
            emit_attn(len(steps) - 1)
            emit_cproj(10**9, limit=10**9, scalar_drain=True)

        for free in reversed(frees):
            free()


class TileCtx:
    """Thin helper so _build can use `tc.tile` / `tc.tile_pool` uniformly."""

    def __init__(self, tile_mod, nc):
        self._tc = tile_mod.TileContext(nc)

    def __enter__(self):
        self._tc.__enter__()
        return self._tc

    def __exit__(self, *exc):
        return self._tc.__exit__(*exc)


def _shard_inputs(inputs):
    import ml_dtypes

    bf = ml_dtypes.bfloat16
    xT = np.ascontiguousarray(
        np.asarray(inputs["hidden_states"], dtype=np.float32).reshape(BS, D).T
    ).astype(bf)
    Wa = np.asarray(inputs["c_attn_w"], dtype=np.float32)
    ba = np.asarray(inputs["c_attn_b"], dtype=np.float32)
    Wp = np.asarray(inputs["c_proj_w"], dtype=np.float32)
    proj = np.asarray(inputs["projectors"], dtype=np.float32)

    in_maps = []
    F = HPC * HD
    for c in range(N_CORES):
        sl = slice(c * F, (c + 1) * F)
        wtil = np.einsum(
            "hde,hef->hdf",
            proj[HPC * c : HPC * (c + 1)],
            Wp[sl, :].reshape(HPC, HD, D),
        ).reshape(F, D)
        in_maps.append(
            {
                "xT": xT,
                "w_qk": np.ascontiguousarray(
                    np.concatenate(
                        [Wa[:, sl], Wa[:, D + c * F : D + (c + 1) * F]], axis=1
                    )
                ).astype(bf),
                "w_v": np.ascontiguousarray(
                    Wa[:, 2 * D + c * F : 2 * D + (c + 1) * F]
                ).astype(bf),
                "b_qk": np.ascontiguousarray(
                    np.concatenate([ba[sl], ba[D + c * F : D + (c + 1) * F]])
                ),
                "b_v512": np.ascontiguousarray(
                    np.tile(ba[2 * D + c * F : 2 * D + (c + 1) * F], 4)
                ),
                "w_til": np.ascontiguousarray(wtil).astype(bf),
            }
        )
    return in_maps


def _get_nc():
    if "nc" not in _CACHE:
        from concourse import bacc

        nc = bacc.Bacc("TRN2", debug=False, num_devices=N_CORES)
        _build(nc)
        nc.compile()
        _CACHE["nc"] = nc
    return _CACHE["nc"]


def _run(inputs, trace=False, trace_kwargs=None):
    from concourse.bass_utils import run_bass_kernel_spmd

    nc = _get_nc()
    in_maps = _shard_inputs(inputs)
    res = run_bass_kernel_spmd(
        nc,
        in_maps,
        core_ids=list(range(N_CORES)),
        trace=trace,
        **(trace_kwargs or {}),
    )
    acc = np.zeros((8, 128, BS), dtype=np.float32)
    for r in res.results:
        acc += np.asarray(r["outT"], dtype=np.float32)
    bp = np.asarray(inputs["c_proj_b"], dtype=np.float32)
    out = acc.transpose(2, 0, 1).reshape(BS, D) + bp[None, :]
    return np.ascontiguousarray(out.reshape(B, S, D)), res


def kernel(**inputs) -> np.ndarray:
    out, _ = _run(inputs, trace=False)
    return out


def simulate_core(inputs, core=0):
    """CoreSim one core's program (for correctness debugging). Returns outT."""
    from concourse.bass_interp import CoreSim

    nc = _get_nc()
    in_maps = _shard_inputs(inputs)
    sim = CoreSim(nc, trace=False)
    for name, arr in in_maps[core].items():
        sim.tensor(name)[:] = arr
    sim.simulate()
    return np.array(sim.tensor("outT"))


# revision 12
# speedup vs baseline: 1.4618x; 1.1622x over previous
# Trainium2 Bass kernel for nn_CompressedGPT2Attention.
#
# Model: B=2, S=2048, D=1024, H=16 heads of HD=64.
#   qkv = x @ c_attn_w + c_attn_b ; causal attention per head;
#   per-head symmetric projector on the attention output; out = attn @ c_proj_w + b.
#
# Sharding (megatron-style tensor parallel over heads, 8 cores x 2 heads):
#   - every core gets the full hidden_states
#   - c_attn (q,k,v) columns sharded by head; the per-head projector is folded
#     into c_proj on the host (W~_h = proj_h @ c_proj_rows_h)
#   - each core writes a full-shape bf16 partial; host does the f32 all-reduce
#     + bias.
#
# v3 structure (vs the 227us v2):
#   - qkv (phase 1) is interleaved INTO the attention step loop: sb0-3 are
#     emitted up front, sb4-7's q/k/v chains are injected between early
#     attention steps, so the PE never serializes a qkv-only prologue while
#     ScalarE idles
#   - one shared [128,512] PSUM pool serves qkv chains AND c_proj outputs
#     (2 banks) + scores 2x[128,1024] (4) + attn (1) + sums (1) = 8 banks
#   - softmax reciprocal on DVE RECIPROCAL for big blocks; ScalarE ln+exp
#     for the qt=0 blocks and the final block (keeps sums-bank hold short
#     where block gaps are tight)
#   - un-normalized attn is drained by ScalarE (Identity ACT) to free the
#     attn PSUM bank fast; normalization mult is bf16 x bf16 (2x DVE mode)
#   - c_proj matmuls are released >=4 steps after their block ends so they
#     never stall on the recip chain; drains on DVE (ScalarE in the tail)
#   - dummy exp at t=0 preloads the ACT table set during the qkv ramp

import numpy as np

B, S, D, H, HD = 2, 2048, 1024, 16, 64
BS = B * S
N_CORES = 8
HPC = H // N_CORES  # heads per core = 2

_CACHE = {}


def _build(nc):
    import concourse.bass as bass
    import concourse.mybir as mybir
    import concourse.tile as tile
    from contextlib import ExitStack

    f32 = mybir.dt.float32
    bf16 = mybir.dt.bfloat16
    AF = mybir.ActivationFunctionType
    OP = mybir.AluOpType

    x_d = nc.dram_tensor("xT", [D, BS], bf16, kind="ExternalInput").ap()
    wqk_d = nc.dram_tensor("w_qk", [D, 2 * HPC * HD], bf16, kind="ExternalInput").ap()
    wv_d = nc.dram_tensor("w_v", [D, HPC * HD], bf16, kind="ExternalInput").ap()
    bqk_d = nc.dram_tensor("b_qk", [2 * HPC * HD], f32, kind="ExternalInput").ap()
    bv_d = nc.dram_tensor("b_v512", [512], f32, kind="ExternalInput").ap()
    wtil_d = nc.dram_tensor("w_til", [HPC * HD, D], bf16, kind="ExternalInput").ap()
    out_d = nc.dram_tensor("outT", [8, 128, BS], bf16, kind="ExternalOutput").ap()

    F = HPC * HD  # 128 features (2 heads stacked)
    NB = BS // 512
    KT = D // 128

    with TileCtx(tile, nc) as tc:
        frees = []

        def ptile(shape, dtype, name):
            t, free = tc.tile(shape, dtype, name=name)
            frees.append(free)
            return t

        qT = ptile([128, BS], bf16, "qT")
        kTt = ptile([128, BS], bf16, "kTt")
        v_s = ptile([128, BS // 128, 128], bf16, "v_s")
        wqk_sb = ptile([128, KT, 2 * F], bf16, "wqk_sb")
        wv_sb = ptile([128, KT, F], bf16, "wv_sb")
        wtil_sb = ptile([128, D], bf16, "wtil_sb")
        bqk_sb = ptile([128, 2], f32, "bqk_sb")
        ones_w = ptile([128, 64], bf16, "ones_w")
        ones_row = ptile([1, 128], bf16, "ones_row")
        bv16 = ptile([1, 512], bf16, "bv16")
        bias_v_bc = ptile([128, 512], f32, "bias_v_bc")
        dummy = ptile([1, 2], f32, "dummy")
        cpr = [ptile([128, 512], bf16, f"cpr{i}") for i in range(NB)]

        nc.any.memset(ones_w[:], 1.0)
        nc.any.memset(ones_row[:], 1.0)
        nc.any.memset(dummy[:], 1.0)
        # preload the exp/ln ACT table set while the PE ramps on qkv
        nc.scalar.activation(dummy[:, 0:1], dummy[:, 1:2], AF.Exp)
        nc.scalar.activation(dummy[:, 0:1], dummy[:, 1:2], AF.Ln)

        nc.sync.dma_start(wqk_sb[:], wqk_d.rearrange("(kt p) f -> p kt f", p=128))
        nc.sync.dma_start(wv_sb[:], wv_d.rearrange("(kt p) f -> p kt f", p=128))
        nc.sync.dma_start(bqk_sb[:], bqk_d.rearrange("(t p) -> p t", p=128))
        nc.gpsimd.dma_start(bv16[:], bv_d[None, :])

        with ExitStack() as ctx:
            ch_pool = ctx.enter_context(tc.tile_pool(name="xchunk", bufs=2))
            sh_ps = ctx.enter_context(tc.tile_pool(name="sh_ps", bufs=2, space="PSUM"))
            sc_ps = ctx.enter_context(tc.tile_pool(name="sc_ps", bufs=2, space="PSUM"))
            attn_ps = ctx.enter_context(tc.tile_pool(name="attn_ps", bufs=1, space="PSUM"))
            sums_ps = ctx.enter_context(tc.tile_pool(name="sums_ps", bufs=1, space="PSUM"))
            epool = ctx.enter_context(tc.tile_pool(name="epool", bufs=3))
            upool = ctx.enter_context(tc.tile_pool(name="upool", bufs=2))
            rpool = ctx.enter_context(tc.tile_pool(name="rpool", bufs=2))
            opool = ctx.enter_context(tc.tile_pool(name="opool", bufs=4))

            # v bias broadcast tile [128,512] (rows all = tiled b_v)
            ps_bv = sh_ps.tile([128, 512], f32, tag="sh")
            nc.tensor.matmul(ps_bv[:], ones_row[:], bv16[:], start=True, stop=True)
            nc.vector.tensor_copy(bias_v_bc[:], ps_bv[:])

            chunks = {}

            def emit_chunk_dma(sb):
                chunk = ch_pool.tile([128, KT, 512], bf16, tag="x", name="chunk")
                nc.sync.dma_start(
                    chunk[:],
                    x_d[:, sb * 512 : (sb + 1) * 512].rearrange(
                        "(kt p) s -> p kt s", p=128
                    ),
                )
                chunks[sb] = chunk

            def emit_qk(sb, ft, scalar_drain):
                dest = (qT, kTt)[ft]
                ps = sh_ps.tile([128, 512], f32, tag="sh", name="ps_qk")
                for kt in range(KT):
                    nc.tensor.matmul(
                        ps[:],
                        wqk_sb[:, kt, ft * F : (ft + 1) * F],
                        chunks[sb][:, kt, :],
                        start=(kt == 0),
                        stop=(kt == KT - 1),
                    )
                dsl = dest[:, sb * 512 : (sb + 1) * 512]
                if scalar_drain:
                    nc.scalar.activation(
                        dsl, ps[:], AF.Identity, bias=bqk_sb[:, ft : ft + 1]
                    )
                else:
                    nc.vector.tensor_scalar(
                        dsl, ps[:], bqk_sb[:, ft : ft + 1], None, OP.add
                    )

            def emit_v(sb):
                ps = sh_ps.tile([128, 512], f32, tag="sh", name="ps_v")
                for j in range(4):
                    for kt in range(KT):
                        nc.tensor.matmul(
                            ps[:, j * 128 : (j + 1) * 128],
                            chunks[sb][:, kt, j * 128 : (j + 1) * 128],
                            wv_sb[:, kt, :],
                            start=(kt == 0),
                            stop=(kt == KT - 1),
                            skip_group_check=True,
                        )
                nc.vector.scalar_tensor_tensor(
                    v_s[:, sb * 4 : (sb + 1) * 4, :], ps[:], 1.0, bias_v_bc[:],
                    OP.mult, OP.add,
                )
                chunks.pop(sb)

            # ---- sb0-3 upfront (b=0 data); ScalarE does the qk drains ----
            for sb in range(4):
                emit_chunk_dma(sb)
                emit_qk(sb, 0, True)
                emit_qk(sb, 1, True)
                emit_v(sb)
            # c_proj weights are first needed ~step 5; keep their DMA out of
            # the startup critical path
            nc.sync.dma_start(wtil_sb[:], wtil_d)

            # remaining qkv work, injected between early attention steps
            inject = {}
            for n, sb in enumerate(range(4, 8)):
                base = 6 * n
                inject.setdefault(base, []).append(("dma", sb))
                inject.setdefault(base + 2, []).append(("q", sb))
                inject.setdefault(base + 4, []).append(("k", sb))
                inject.setdefault(base + 6, []).append(("v", sb))

            # ---- attention steps ----
            steps = []
            for b in range(B):
                for qt in range(4):
                    nkj = 4 * (qt + 1)
                    for kj in range(nkj):
                        steps.append((b, qt, kj, kj == 0, kj == nkj - 1))

            state = {}
            pend_cproj = []

            def emit_scores(i):
                b, qt, kj, first, last = steps[i]
                p = kj - 4 * qt
                lo = 128 * max(p, 0)
                qi = b * S + qt * 512
                kjc = b * S + kj * 128
                psc = sc_ps.tile([128, 1024], f32, tag="sc", name="psc")
                nc.tensor.matmul(
                    psc[:, lo:512], kTt[0:64, kjc : kjc + 128],
                    qT[0:64, qi + lo : qi + 512],
                    start=True, stop=True, tile_position=(0, 0),
                )
                nc.tensor.matmul(
                    psc[:, 512 + lo : 1024], kTt[64:128, kjc : kjc + 128],
                    qT[64:128, qi + lo : qi + 512],
                    start=True, stop=True, tile_position=(64, 0),
                )
                e = epool.tile([128, 1024], bf16, tag="e", name="e")
                if p > 0:
                    # columns [0:128p] are fully-masked queries for this kj:
                    # the attn/sums matmuls skip them instead of zero-filling
                    nc.scalar.activation(
                        e[:, 128 * p : 512], psc[:, 128 * p : 512], AF.Exp, scale=0.125
                    )
                    nc.scalar.activation(
                        e[:, 512 + 128 * p : 1024], psc[:, 512 + 128 * p : 1024],
                        AF.Exp, scale=0.125,
                    )
                else:
                    nc.scalar.activation(e[:], psc[:], AF.Exp, scale=0.125)
                if p >= 0:
                    for off in (0, 512):
                        nc.gpsimd.affine_select(
                            e[:, off + 128 * p : off + 128 * (p + 1)],
                            e[:, off + 128 * p : off + 128 * (p + 1)],
                            pattern=[[1, 128]], base=0,
                            channel_multiplier=-1,
                            compare_op=OP.is_ge, fill=0.0,
                        )
                state[i] = e

            def emit_attn(i):
                b, qt, kj, first, last = steps[i]
                p = kj - 4 * qt
                lo = 128 * max(p, 0)
                e = state.pop(i)
                if first:
                    state["attn"] = attn_ps.tile([128, 512], f32, tag="attn", name="ps_attn")
                    state["sums"] = sums_ps.tile([128, 512], f32, tag="sums", name="ps_sums")
                ps_attn, ps_sums = state["attn"], state["sums"]
                vs = v_s[:, b * 16 + kj, :]
                eA, eB = e[:, lo:512], e[:, 512 + lo : 1024]
                nc.tensor.matmul(
                    ps_attn[0:64, lo:512], vs[:, 0:64], eA,
                    start=first, stop=last, tile_position=(0, 0),
                    skip_group_check=True,
                )
                nc.tensor.matmul(
                    ps_attn[64:128, lo:512], vs[:, 64:128], eB,
                    start=first, stop=last, tile_position=(0, 64),
                    skip_group_check=True,
                )
                nc.tensor.matmul(
                    ps_sums[0:64, lo:512], ones_w[:, 0:64], eA,
                    start=first, stop=last, tile_position=(0, 0),
                    skip_group_check=True,
                )
                nc.tensor.matmul(
                    ps_sums[64:128, lo:512], ones_w[:, 0:64], eB,
                    start=first, stop=last, tile_position=(0, 64),
                    skip_group_check=True,
                )
                if last:
                    blk = b * 4 + qt
                    ps_attn = state.pop("attn")
                    ps_sums = state.pop("sums")
                    # drain both PSUM banks fast (short holds -> no stall for
                    # the next block's accumulations)
                    unA = upool.tile([128, 512], bf16, tag="u", name="unA")
                    nc.vector.tensor_copy(unA[:], ps_attn[:])
                    rec = rpool.tile([128, 512], bf16, tag="r", name="rec")
                    with nc.allow_low_precision(reason="softmax recip in bf16"):
                        if qt == 0 or blk == NB - 1:
                            # short recip on ScalarE: 1/x = exp(-ln(x))
                            ln_t = rpool.tile([128, 512], f32, tag="r", name="ln_t")
                            nc.scalar.activation(ln_t[:], ps_sums[:], AF.Ln)
                            nc.scalar.activation(rec[:], ln_t[:], AF.Exp, scale=-1.0)
                        else:
                            sums_sb = rpool.tile([128, 512], f32, tag="r", name="sums_sb")
                            nc.vector.tensor_copy(sums_sb[:], ps_sums[:])
                            nc.vector.reciprocal(rec[:], sums_sb[:])
                        nc.vector.tensor_tensor(cpr[blk][:], unA[:], rec[:], OP.mult)
                    for dt in range(8):
                        pend_cproj.append((blk, dt, i + 4))

            def emit_cproj(i, limit=2, scalar_drain=False):
                n = 0
                while pend_cproj and pend_cproj[0][2] <= i and n < limit:
                    blk, dt, _ = pend_cproj.pop(0)
                    pcp = sh_ps.tile([128, 512], f32, tag="sh", name="pcp")
                    nc.tensor.matmul(
                        pcp[:], wtil_sb[:, dt * 128 : (dt + 1) * 128],
                        cpr[blk][:], start=True, stop=True,
                    )
                    ot = opool.tile([128, 512], bf16, tag="ot", name="ot")
                    if scalar_drain and dt % 2 == 0:
                        nc.scalar.activation(ot[:], pcp[:], AF.Identity)
                    else:
                        nc.vector.tensor_copy(ot[:], pcp[:])
                    nc.sync.dma_start(
                        out_d[dt][:, blk * 512 : (blk + 1) * 512], ot[:]
                    )
                    n += 1

            for i in range(len(steps)):
                emit_scores(i)
                for item in inject.pop(i, []):
                    kind, sb = item
                    if kind == "dma":
                        emit_chunk_dma(sb)
                    elif kind == "q":
                        emit_qk(sb, 0, False)
                    elif kind == "k":
                        emit_qk(sb, 1, False)
                    else:
                        emit_v(sb)
                if i > 0:
                    emit_attn(i - 1)
                emit_cproj(i, limit=2)
            emit_attn(len(steps) - 1)
            emit_cproj(10**9, limit=10**9, scalar_drain=True)

        for free in reversed(frees):
            free()


class TileCtx:
    """Thin helper so _build can use `tc.tile` / `tc.tile_pool` uniformly."""

    def __init__(self, tile_mod, nc):
        self._tc = tile_mod.TileContext(nc)

    def __enter__(self):
        self._tc.__enter__()
        return self._tc

    def __exit__(self, *exc):
        return self._tc.__exit__(*exc)


def _shard_inputs(inputs):
    import ml_dtypes

    bf = ml_dtypes.bfloat16
    xT = np.ascontiguousarray(
        np.asarray(inputs["hidden_states"], dtype=np.float32).reshape(BS, D).T
    ).astype(bf)
    Wa = np.asarray(inputs["c_attn_w"], dtype=np.float32)
    ba = np.asarray(inputs["c_attn_b"], dtype=np.float32)
    Wp = np.asarray(inputs["c_proj_w"], dtype=np.float32)
    proj = np.asarray(inputs["projectors"], dtype=np.float32)

    in_maps = []
    F = HPC * HD
    for c in range(N_CORES):
        sl = slice(c * F, (c + 1) * F)
        wtil = np.einsum(
            "hde,hef->hdf",
            proj[HPC * c : HPC * (c + 1)],
            Wp[sl, :].reshape(HPC, HD, D),
        ).reshape(F, D)
        in_maps.append(
            {
                "xT": xT,
                "w_qk": np.ascontiguousarray(
                    np.concatenate(
                        [Wa[:, sl], Wa[:, D + c * F : D + (c + 1) * F]], axis=1
                    )
                ).astype(bf),
                "w_v": np.ascontiguousarray(
                    Wa[:, 2 * D + c * F : 2 * D + (c + 1) * F]
                ).astype(bf),
                "b_qk": np.ascontiguousarray(
                    np.concatenate([ba[sl], ba[D + c * F : D + (c + 1) * F]])
                ),
                "b_v512": np.ascontiguousarray(
                    np.tile(ba[2 * D + c * F : 2 * D + (c + 1) * F], 4)
                ),
                "w_til": np.ascontiguousarray(wtil).astype(bf),
            }
        )
    return in_maps


def _get_nc():
    if "nc" not in _CACHE:
        from concourse import bacc

        nc = bacc.Bacc("TRN2", debug=False, num_devices=N_CORES)
        _build(nc)
        nc.compile()
        _CACHE["nc"] = nc
    return _CACHE["nc"]


def _run(inputs, trace=False, trace_kwargs=None):
    from concourse.bass_utils import run_bass_kernel_spmd

    nc = _get_nc()
    in_maps = _shard_inputs(inputs)
    res = run_bass_kernel_spmd(
        nc,
        in_maps,
        core_ids=list(range(N_CORES)),
        trace=trace,
        **(trace_kwargs or {}),
    )
    acc = np.zeros((8, 128, BS), dtype=np.float32)
    for r in res.results:
        acc += np.asarray(r["outT"], dtype=np.float32)
    bp = np.asarray(inputs["c_proj_b"], dtype=np.float32)
    out = acc.transpose(2, 0, 1).reshape(BS, D) + bp[None, :]
    return np.ascontiguousarray(out.reshape(B, S, D)), res


def kernel(**inputs) -> np.ndarray:
    out, _ = _run(inputs, trace=False)
    return out


def simulate_core(inputs, core=0):
    """CoreSim one core's program (for correctness debugging). Returns outT."""
    from concourse.bass_interp import CoreSim

    nc = _get_nc()
    in_maps = _shard_inputs(inputs)
    sim = CoreSim(nc, trace=False)
    for name, arr in in_maps[core].items():
        sim.tensor(name)[:] = arr
    sim.simulate()
    return np.array(sim.tensor("outT"))
